# revision 19
# baseline (speedup 1.0000x reference)
# Trainium2 Bass kernel for nn_DiT_11536282157395 (4-layer DiT block,
# B=1 S=1024 D=1024 H=16 DFF=4096, frame-sparse attention).
#
# Sharding: sequence-parallel over 8 NeuronCores, 128 tokens/core.
# Residual kept token-major in SBUF; per-layer weights streamed from HBM
# in bf16. Attention computed with transposed scores (keys on psum
# partitions) so no probs transpose is needed; softmax without
# max-subtraction (scores are bounded: rms-normed q/k, 1/sqrt(dh) folded
# into the k rope tables). K/V exchanged once per layer with a single
# 8-rank AllGather; local-window layers read only the left-neighbor
# block back via a partition-id-indexed DMA.
import sys
import os

sys.path.insert(0, '/opt/trn_rl_repo')

import numpy as np
import ml_dtypes

import concourse.bass as bass
import concourse.bacc as bacc
import concourse.mybir as mybir
import concourse.tile as tile
from concourse import bass_utils
from concourse.alu_op_type import AluOpType
from concourse.masks import make_identity

B, S, D, H, L = 1, 1024, 1024, 16, 4
DH = D // H          # 64
HALF = DH // 2       # 32
TPF = 8
LOCAL_W = 8
DFF = 4 * D
NC_N = 8
T = S // NC_N        # 128 tokens per core
DC = D // 128        # 8 d-chunks
FC = DFF // 128      # 32 dff-chunks
MODC = 6 * D         # mod columns per layer (shift1|sc1|g1|shift2|sc2|g2)
MODS = L * MODC // NC_N   # 3072 mod columns per core shard
KV_SLOT = D * T * 2  # 262144 bf16 elems per rank slot (kT + v)

F32 = mybir.dt.float32
BF16 = mybir.dt.bfloat16
AF = mybir.ActivationFunctionType
AX = mybir.AxisListType

_CACHE = {}


def build(n_layers=L, debug=False):
    nc = bacc.Bacc("TRN2", target_bir_lowering=False, debug=False, num_devices=NC_N)

    def din(name, shape, dt=BF16):
        return nc.dram_tensor(name, shape, dt, kind="ExternalInput").ap()

    tens = dict(
        x_in=din("x_in", [T, D], F32),
        cond_t=din("cond_t", [DC, 128, 1], BF16),
        w_mod=din("w_mod", [DC, 128, MODS], BF16),
        b_mod=din("b_mod", [1, MODS], F32),
        w_qkv=din("w_qkv", [L, DC, 128, 3 * D], BF16),
        w_o=din("w_o", [L, DC, 128, D], BF16),
        w_1=din("w_1", [L, FC, 128, DC * 128], BF16),
        w_2=din("w_2", [L, FC, 128, D], BF16),
        l_slot=din("l_slot", [1, 1], mybir.dt.uint32),
        m_glob=din("m_glob", [NC_N, T, T], F32),
        m_loc=din("m_loc", [2, T, T], F32),
        rope_t=din("rope_t", [4, T, HALF], F32),
        x_out=nc.dram_tensor("x_out", [T, D], F32, kind="ExternalOutput").ap(),
    )
    tens["n_layers"] = n_layers
    tens["debug"] = debug
    if debug:
        for nm in ("dbg_hT", "dbg_qn", "dbg_kn", "dbg_v", "dbg_osb", "dbg_x1",
                   "dbg_kTl", "dbg_vl", "dbg_oT", "dbg_att", "dbg_g1"):
            tens[nm] = nc.dram_tensor(nm, [T, D], F32, kind="ExternalOutput").ap()
        tens["dbg_e0"] = nc.dram_tensor("dbg_e0", [T, NC_N * T], F32,
                                        kind="ExternalOutput").ap()

    import contextlib
    with tile.TileContext(nc) as tc:
        with contextlib.ExitStack() as ctx:
            _build_body(nc, tc, ctx, tens)
    nc.compile()
    return nc


def _build_body(nc, tc, ctx, t_):
    l_slot = t_["l_slot"]
    x_in, cond_t, w_mod, b_mod = t_["x_in"], t_["cond_t"], t_["w_mod"], t_["b_mod"]
    w_qkv, w_o, w_1, w_2 = t_["w_qkv"], t_["w_o"], t_["w_1"], t_["w_2"]
    m_glob, m_loc, rope_t, x_out = t_["m_glob"], t_["m_loc"], t_["rope_t"], t_["x_out"]

    P = 128
    pool1 = ctx.enter_context(tc.tile_pool(name="persist", bufs=1))
    poolw = ctx.enter_context(tc.tile_pool(name="wstream", bufs=2))
    poolt = ctx.enter_context(tc.tile_pool(name="tmp", bufs=2))
    poole = ctx.enter_context(tc.tile_pool(name="exp", bufs=4))
    psum = ctx.enter_context(tc.tile_pool(name="ps", bufs=8, space="PSUM"))
    dram = ctx.enter_context(tc.tile_pool(name="dram", bufs=1, space="DRAM"))

    def pst(cols=512, dt=F32, name="ps"):
        return psum.tile([P, cols], dt, name=name, tag="ps", bufs=8)

    _lreg = nc.alloc_registers("lslot_reg", mybir.ALL_ENGINES)
    nc.regs_load(_lreg, l_slot[0:1, 0:1])
    lslot = nc.snap(_lreg, donate=True, min_val=0, max_val=NC_N - 1)

    # ---- constants ----
    ident = pool1.tile([P, P], BF16)
    make_identity(nc, ident[:])
    ones_b = pool1.tile([P, 1], BF16)
    nc.vector.memset(ones_b[:], 1.0)
    eps_ln = pool1.tile([P, 1], F32)
    nc.vector.memset(eps_ln[:], 1e-5)
    eps_rms = pool1.tile([P, 1], F32)
    nc.vector.memset(eps_rms[:], 1e-6)

    ropes = pool1.tile([P, 4, HALF], F32, name="ropes")
    nc.sync.dma_start(ropes[:], rope_t.rearrange("r t h -> t r h"))
    cosq, sinq = ropes[:, 0, :], ropes[:, 1, :]
    cosk, sink = ropes[:, 2, :], ropes[:, 3, :]

    mglob_sb = pool1.tile([P, NC_N, T], F32, name="mglob")
    nc.sync.dma_start(mglob_sb[:], m_glob.rearrange("r k q -> k r q"))
    mloc_sb = pool1.tile([P, 2, T], F32, name="mloc")
    nc.sync.dma_start(mloc_sb[:], m_loc.rearrange("r k q -> k r q"))

    # ---- residual ----
    x_sb = pool1.tile([P, D], F32, name="x_sb")
    nc.sync.dma_start(x_sb[:], x_in[:, :])

    # ---- modulation phase ----
    cond_sb = pool1.tile([P, DC], BF16, name="cond_sb")
    nc.sync.dma_start(cond_sb[:], cond_t.rearrange("i p o -> p (i o)"))
    NMOD = MODS // 512  # 6
    mod_ps = [psum.tile([1, 512], F32, name=f"mod_ps{j}", tag="ps", bufs=8)
              for j in range(NMOD)]
    for i in range(DC):
        wm = poolw.tile([P, MODS], BF16, name="wm", tag="wq")
        nc.sync.dma_start(wm[:], w_mod[i, :, :])
        for j in range(NMOD):
            nc.tensor.matmul(mod_ps[j][:], cond_sb[:, i:i + 1],
                             wm[:, j * 512:(j + 1) * 512],
                             start=(i == 0), stop=(i == DC - 1))
    mod_sb = pool1.tile([1, MODS], F32, name="mod_sb")
    bmod_sb = pool1.tile([1, MODS], F32, name="bmod_sb")
    nc.sync.dma_start(bmod_sb[:], b_mod[:, :])
    for j in range(NMOD):
        nc.vector.tensor_add(mod_sb[:, j * 512:(j + 1) * 512], mod_ps[j][:],
                             bmod_sb[:, j * 512:(j + 1) * 512])
    mod_in = dram.tile([1, MODS], F32, name="mod_in")
    modg = dram.tile([1, L * MODC], F32, name="modg", addr_space="Shared")
    nc.sync.dma_start(mod_in[:], mod_sb[:])
    nc.gpsimd.collective_compute(
        "AllGather", AluOpType.bypass,
        replica_groups=[list(range(NC_N))],
        ins=[mod_in.opt()], outs=[modg.opt()])

    kv_g8s = [dram.tile([1, NC_N * KV_SLOT], BF16, name=f"kv_g8_{l}",
                        addr_space="Shared") for l in range(L)]

    def bcast_vec(off):
        g = poolt.tile([P, D], F32, name="gvec", tag="gvec", bufs=2)
        src = bass.AP(tensor=modg.tensor, offset=modg.offset + off,
                      ap=[[0, P], [1, D]])
        nc.sync.dma_start(g[:], src)
        return g

    def modvec(off):
        v = poolt.tile([P, DC], F32, name="mv", tag="mv", bufs=6)
        src = bass.AP(tensor=modg.tensor, offset=modg.offset + off,
                      ap=[[1, P], [P, DC]])
        nc.sync.dma_start(v[:], src)
        return v

    def layer_norm_mod(lyr, vec_off):
        # returns hT: [128(d in chunk), DC, 128 tok] bf16 modulated LN of x_sb
        shift = modvec(lyr * MODC + vec_off * D)
        scale = modvec(lyr * MODC + (vec_off + 1) * D)
        scale1 = poolt.tile([P, DC], F32, name="sc1p", tag="sc1p", bufs=2)
        nc.vector.tensor_scalar_add(scale1[:], scale[:], 1.0)

        sums = poolt.tile([P, 1], F32, name="lnsum", tag="lnsum")
        nc.vector.reduce_sum(sums[:], x_sb[:], AX.X)
        xsq = poolt.tile([P, D], BF16, name="xsq", tag="sqs")
        sqs = poolt.tile([P, 1], F32, name="lnsq", tag="lnsq")
        nc.scalar.activation(xsq[:], x_sb[:], AF.Square, accum_out=sqs[:])
        mean = poolt.tile([P, 1], F32, name="lnmean", tag="lnmean")
        nc.vector.tensor_scalar_mul(mean[:], sums[:], 1.0 / D)
        var = poolt.tile([P, 1], F32, name="lnvar", tag="lnvar")
        msq = poolt.tile([P, 1], F32, name="lnmsq", tag="lnmsq")
        nc.vector.tensor_mul(msq[:], mean[:], mean[:])
        nc.vector.tensor_scalar(var[:], sqs[:], 1.0 / D, None, AluOpType.mult)
        nc.vector.tensor_sub(var[:], var[:], msq[:])
        rstd = poolt.tile([P, 1], F32, name="lnrstd", tag="lnrstd")
        nc.scalar.activation(rstd[:], var[:], AF.Sqrt, bias=eps_ln[:])
        nc.vector.reciprocal(rstd[:], rstd[:])
        nmr = poolt.tile([P, 1], F32, name="lnnmr", tag="lnnmr")
        nc.vector.tensor_mul(nmr[:], mean[:], rstd[:])
        nc.vector.tensor_scalar_mul(nmr[:], nmr[:], -1.0)
        xn = poolt.tile([P, D], BF16, name="xn", tag="xn")
        nc.scalar.activation(xn[:], x_sb[:], AF.Identity, bias=nmr[:], scale=rstd[:])

        hT = poolt.tile([P, DC, T], BF16, name="hT", tag="hT", bufs=2)
        for i in range(DC):
            tp = pst(P, BF16, name="tps")
            nc.tensor.transpose(tp[:], xn[:, i * P:(i + 1) * P], ident[:])
            nc.scalar.activation(hT[:, i, :], tp[:], AF.Identity,
                                 bias=shift[:, i:i + 1], scale=scale1[:, i:i + 1])
        return hT

    def rope_rms(qk_ps, qn, tabc, tabs):
        # qk_ps: 2 psum tiles [128, 512] (= [tok, 1024] of q or k); qn: out bf16
        sq = poolt.tile([P, D], BF16, name="rsq", tag="sqs")
        for j in range(2):
            nc.scalar.activation(sq[:, j * 512:(j + 1) * 512], qk_ps[j][:], AF.Square)
        s2 = poolt.tile([P, H], F32, name="rs2", tag="rs2")
        nc.vector.reduce_sum(s2[:], sq.rearrange("p (h d) -> p h d", h=H), AX.X)
        rstd = poolt.tile([P, H], F32, name="rrstd", tag="rrstd")
        nc.vector.tensor_scalar_mul(s2[:], s2[:], 1.0 / DH)
        nc.scalar.activation(rstd[:], s2[:], AF.Sqrt, bias=eps_rms[:])
        nc.vector.reciprocal(rstd[:], rstd[:])
        tmp = poolt.tile([P, DH], F32, name="rtmp", tag="rtmp")
        for h in range(H):
            ps = qk_ps[h // 8]
            c0 = (h % 8) * DH
            x1, x2 = ps[:, c0:c0 + HALF], ps[:, c0 + HALF:c0 + DH]
            t1, t2 = tmp[:, :HALF], tmp[:, HALF:]
            nc.vector.tensor_mul(t1, x1, tabc)
            nc.vector.tensor_mul(t2, x2, tabs)
            nc.vector.tensor_sub(t1, t1, t2)
            nc.vector.tensor_scalar_mul(qn[:, h * DH:h * DH + HALF], t1,
                                        rstd[:, h:h + 1])
            nc.vector.tensor_mul(t1, x1, tabs)
            nc.vector.tensor_mul(t2, x2, tabc)
            nc.vector.tensor_add(t1, t1, t2)
            nc.vector.tensor_scalar_mul(qn[:, h * DH + HALF:(h + 1) * DH], t1,
                                        rstd[:, h:h + 1])

    def transpose8(src_sb, dst):
        for i in range(DC):
            tp = pst(P, BF16, name="tpt")
            nc.tensor.transpose(tp[:], src_sb[:, i * P:(i + 1) * P], ident[:])
            nc.vector.tensor_copy(dst[:, i, :], tp[:])

    # ================= layers =================
    debug = t_["debug"]

    def dump(name, sb):
        if debug:
            f = poolt.tile([P, D], F32, name="dbgf", tag="dbgf", bufs=2)
            nc.vector.tensor_copy(f[:], sb)
            nc.sync.dma_start(t_[name][:, :], f[:])

    for lyr in range(t_["n_layers"]):
        is_glob = (lyr % 4 == 0)
        dbg0 = debug and lyr == t_["n_layers"] - 1

        # ---- attention sub-block ----
        hT = layer_norm_mod(lyr, 0)

        qkv_ps = [pst(name=f"qkv{j}") for j in range(6)]
        for i in range(DC):
            wq = poolw.tile([P, 3 * D], BF16, name="wq", tag="wq")
            nc.sync.dma_start(wq[:], w_qkv[lyr, i, :, :])
            for j in range(6):
                nc.tensor.matmul(qkv_ps[j][:], hT[:, i, :],
                                 wq[:, j * 512:(j + 1) * 512],
                                 start=(i == 0), stop=(i == DC - 1))

        v_sb = poolt.tile([P, D], BF16, name="v_sb", tag="v_sb", bufs=2)
        for j in range(2):
            nc.scalar.activation(v_sb[:, j * 512:(j + 1) * 512], qkv_ps[4 + j][:],
                                 AF.Copy)
        qn = poolt.tile([P, D], BF16, name="qn", tag="qn")
        kn = poolt.tile([P, D], BF16, name="kn", tag="kn")
        rope_rms(qkv_ps[0:2], qn, cosq, sinq)
        rope_rms(qkv_ps[2:4], kn, cosk, sink)

        if dbg0:
            dump("dbg_hT", hT.rearrange("p i t -> p (i t)"))
            dump("dbg_qn", qn[:])
            dump("dbg_kn", kn[:])
            dump("dbg_v", v_sb[:])
        qT = poolt.tile([P, DC, T], BF16, name="qT", tag="qT")
        kT = poolt.tile([P, DC, T], BF16, name="kT", tag="kT")
        transpose8(qn, qT)
        transpose8(kn, kT)

        # ---- kv AllGather (kT | v), 9-slot layout ----
        kv_g9 = kv_g8s[lyr]
        kv_in = dram.tile([1, KV_SLOT], BF16, name="kv_in", tag="kv_in", bufs=2)
        for i in range(DC):
            nc.sync.dma_start(
                kv_in[:, i * (P * T):(i + 1) * (P * T)].rearrange(
                    "o (p t) -> (o p) t", p=P), kT[:, i, :])
        nc.sync.dma_start(
            kv_in[:, D * T:].rearrange("o (t f) -> (o t) f", t=P), v_sb[:])
        nc.gpsimd.collective_compute(
            "AllGather", AluOpType.bypass,
            replica_groups=[list(range(NC_N))],
            ins=[kv_in.opt()],
            outs=[kv_g9.opt()])

        # ---- fetch needed key chunks ----
        if is_glob:
            kTg = pool1.tile([P, NC_N, DC, T], BF16, name=f"kTg{lyr}")
            vg = pool1.tile([P, NC_N, D], BF16, name=f"vg{lyr}")
            for r in range(NC_N):
                base = r * KV_SLOT
                nc.sync.dma_start(
                    kTg[:, r, :, :],
                    kv_g9[:, base:base + D * T].rearrange(
                        "o (i p t) -> (o p) i t", p=P, i=DC))
                nc.sync.dma_start(
                    vg[:, r, :],
                    kv_g9[:, base + D * T:base + KV_SLOT].rearrange(
                        "o (t f) -> (o t) f", t=P))
            chunks = [(lambda h, r=r: kTg[(h % 2) * DH:(h % 2 + 1) * DH, r, h // 2, :],
                       lambda h, r=r: vg[:, r, h * DH:(h + 1) * DH],
                       mglob_sb[:, r, :]) for r in range(NC_N)]
        else:
            kTl = poolt.tile([P, DC, T], BF16, name="kTl", tag="kTl")
            vl = poolt.tile([P, D], BF16, name="vl", tag="vl")
            nc.sync.dma_start(
                kTl[:],
                kv_g9[:, bass.ds(lslot * KV_SLOT, D * T)].rearrange(
                    "o (i p t) -> (o p) i t", p=P, i=DC))
            nc.sync.dma_start(
                vl[:],
                kv_g9[:, bass.ds(lslot * KV_SLOT + D * T, D * T)].rearrange(
                    "o (t f) -> (o t) f", t=P))
            chunks = [(lambda h: kTl[(h % 2) * DH:(h % 2 + 1) * DH, h // 2, :],
                       lambda h: vl[:, h * DH:(h + 1) * DH],
                       mloc_sb[:, 0, :]),
                      (lambda h: kT[(h % 2) * DH:(h % 2 + 1) * DH, h // 2, :],
                       lambda h: v_sb[:, h * DH:(h + 1) * DH],
                       mloc_sb[:, 1, :])]

        # ---- attention core ----
        o_sb = poolt.tile([P, D], BF16, name="o_sb", tag="o_sb")
        nchunks = len(chunks)
        for h in range(H):
            o_ps = pst(DH, name="o_ps")
            su_ps = psum.tile([P, 1], F32, name="su_ps", tag="ps", bufs=8)
            for ci, (ktf, vf, msk) in enumerate(chunks):
                s_ps = pst(T, name="s_ps")
                qslice = qT[(h % 2) * DH:(h % 2 + 1) * DH, h // 2, :]
                nc.tensor.matmul(s_ps[:], ktf(h), qslice, start=True, stop=True)
                nc.vector.tensor_add(s_ps[:], s_ps[:], msk)
                e_sb = poole.tile([P, T], BF16, name="e_sb", tag="e_sb")
                nc.scalar.activation(e_sb[:], s_ps[:], AF.Exp)
                if dbg0 and h == 0:
                    ef = poolt.tile([P, T], F32, name="dbge", tag="dbgf")
                    nc.vector.tensor_copy(ef[:], e_sb[:])
                    nc.sync.dma_start(t_["dbg_e0"][:, ci * T:(ci + 1) * T], ef[:])
                nc.tensor.matmul(o_ps[:], e_sb[:], vf(h),
                                 start=(ci == 0), stop=(ci == nchunks - 1))
                nc.tensor.matmul(su_ps[:], e_sb[:], ones_b[:],
                                 start=(ci == 0), stop=(ci == nchunks - 1))
            r_sb = poolt.tile([P, 1], F32, name="r_sb", tag="r_sb", bufs=4)
            nc.vector.reciprocal(r_sb[:], su_ps[:])
            nc.vector.tensor_scalar_mul(o_sb[:, h * DH:(h + 1) * DH],
                                        o_ps[:], r_sb[:])

        if dbg0:
            dump("dbg_osb", o_sb[:])
            if not is_glob:
                dump("dbg_kTl", kTl.rearrange("p i t -> p (i t)"))
                dump("dbg_vl", vl[:])
        oT = poolt.tile([P, DC, T], BF16, name="oT", tag="oT")
        transpose8(o_sb, oT)
        if dbg0:
            dump("dbg_oT", oT.rearrange("p i t -> p (i t)"))

        att_ps = [pst(name=f"att{j}") for j in range(2)]
        for i in range(DC):
            wo = poolw.tile([P, D], BF16, name="wo", tag="wo")
            nc.sync.dma_start(wo[:], w_o[lyr, i, :, :])
            for j in range(2):
                nc.tensor.matmul(att_ps[j][:], oT[:, i, :],
                                 wo[:, j * 512:(j + 1) * 512],
                                 start=(i == 0), stop=(i == DC - 1))
        g1 = bcast_vec(lyr * MODC + 2 * D)
        if dbg0:
            dump("dbg_g1", g1[:])
            af = poolt.tile([P, D], F32, name="dbga", tag="dbgf")
            for j in range(2):
                nc.vector.tensor_copy(af[:, j * 512:(j + 1) * 512], att_ps[j][:])
            nc.sync.dma_start(t_["dbg_att"][:, :], af[:])
        for j in range(2):
            sl = slice(j * 512, (j + 1) * 512)
            tt = poolt.tile([P, 512], F32, name="resid", tag="resid")
            nc.vector.tensor_mul(tt[:], att_ps[j][:], g1[:, sl])
            nc.vector.tensor_add(x_sb[:, sl], x_sb[:, sl], tt[:])

        if dbg0:
            dump("dbg_x1", x_sb[:])

        # ---- mlp sub-block ----
        h2T = layer_norm_mod(lyr, 3)
        out_ps = [pst(name=f"mlp{j}") for j in range(2)]
        for f in range(FC):
            w1t = poolw.tile([P, DC * 128], BF16, name="w1t", tag="w1t")
            nc.sync.dma_start(w1t[:], w_1[lyr, f, :, :])
            hid_ps = pst(P, name="hid_ps")
            for i in range(DC):
                nc.tensor.matmul(hid_ps[:], w1t[:, i * P:(i + 1) * P], h2T[:, i, :],
                                 start=(i == 0), stop=(i == DC - 1))
            hid_sb = poole.tile([P, P], BF16, name="hid_sb", tag="hid_sb")
            nc.scalar.activation(hid_sb[:], hid_ps[:], AF.Gelu_apprx_tanh)
            w2t = poolw.tile([P, D], BF16, name="w2t", tag="w2t")
            nc.sync.dma_start(w2t[:], w_2[lyr, f, :, :])
            for j in range(2):
                nc.tensor.matmul(out_ps[j][:], hid_sb[:],
                                 w2t[:, j * 512:(j + 1) * 512],
                                 start=(f == 0), stop=(f == FC - 1))
        g2 = bcast_vec(lyr * MODC + 5 * D)
        for j in range(2):
            sl = slice(j * 512, (j + 1) * 512)
            tt = poolt.tile([P, 512], F32, name="resid2", tag="resid")
            nc.vector.tensor_mul(tt[:], out_ps[j][:], g2[:, sl])
            nc.vector.tensor_add(x_sb[:, sl], x_sb[:, sl], tt[:])

    nc.sync.dma_start(x_out[:, :], x_sb[:])


# ======================= host side =======================

def _bf16(a):
    return np.asarray(a, np.float32).astype(ml_dtypes.bfloat16)


def _frame_mask_T(wind, qpos, kpos):
    fq = (qpos // TPF)[None, :]
    fk = (kpos // TPF)[:, None]
    ok = (fk <= fq) & ((fq - fk) < wind)
    return np.where(ok, 0.0, -1e30).astype(np.float32)


def _prep_inputs(inputs):
    x = np.asarray(inputs['x'], np.float32)[0]
    cond = np.asarray(inputs['cond'], np.float32)[0]
    qkv_w = np.asarray(inputs['qkv_w'], np.float32)
    out_w = np.asarray(inputs['out_w'], np.float32)
    mlp_w1 = np.asarray(inputs['mlp_w1'], np.float32)
    mlp_w2 = np.asarray(inputs['mlp_w2'], np.float32)
    for k in ('qkv_b', 'out_b', 'mlp_b1', 'mlp_b2', 'ada1_b', 'ada2_b'):
        assert not np.any(np.asarray(inputs[k])), f"{k} nonzero; kernel folds it as zero"

    wq = _bf16(qkv_w.reshape(L, DC, 128, 3 * D))
    wo = _bf16(out_w.reshape(L, DC, 128, D))
    w1 = _bf16(np.ascontiguousarray(
        mlp_w1.reshape(L, DC, 128, FC, 128).transpose(0, 3, 2, 1, 4)
        .reshape(L, FC, 128, DC * 128)))
    w2 = _bf16(mlp_w2.reshape(L, FC, 128, D))

    wm_l, bm_l = [], []
    for i in range(L):
        wm_l.append(np.concatenate([
            np.asarray(inputs['ada1_w'], np.float32)[i],
            np.asarray(inputs['gate1_w'], np.float32)[i],
            np.asarray(inputs['ada2_w'], np.float32)[i],
            np.asarray(inputs['gate2_w'], np.float32)[i]], axis=1))
        bm_l.append(np.concatenate([
            np.asarray(inputs['ada1_b'], np.float32)[i],
            np.asarray(inputs['gate1_b'], np.float32)[i],
            np.asarray(inputs['ada2_b'], np.float32)[i],
            np.asarray(inputs['gate2_b'], np.float32)[i]], axis=0))
    wm_all = np.concatenate(wm_l, axis=1)       # [D, L*6D]
    bm_all = np.concatenate(bm_l, axis=0)       # [L*6D]

    condt = _bf16(cond.reshape(DC, 128, 1))

    half = HALF
    inv = 1.0 / (10000.0 ** (np.arange(half, dtype=np.float32) / half))

    in_maps = []
    for c in range(NC_N):
        pos = np.arange(c * T, (c + 1) * T, dtype=np.float32)
        ang = pos[:, None] * inv[None, :]
        cos, sin = np.cos(ang), np.sin(ang)
        scl = 1.0 / np.sqrt(DH)
        rope = np.stack([cos, sin, cos * scl, sin * scl]).astype(np.float32)

        qpos = np.arange(c * T, (c + 1) * T)
        mg = np.stack([_frame_mask_T(128, qpos, np.arange(r * T, (r + 1) * T))
                       for r in range(NC_N)])
        if c > 0:
            ml_halo = _frame_mask_T(LOCAL_W, qpos, np.arange((c - 1) * T, c * T))
        else:
            ml_halo = np.full((T, T), -1e30, np.float32)
        ml_own = _frame_mask_T(LOCAL_W, qpos, qpos)
        ml = np.stack([ml_halo, ml_own])

        shard = slice(c * MODS, (c + 1) * MODS)
        in_maps.append({
            "x_in": np.ascontiguousarray(x[c * T:(c + 1) * T]),
            "cond_t": condt,
            "w_mod": _bf16(np.ascontiguousarray(wm_all[:, shard])
                           .reshape(DC, 128, MODS)),
            "b_mod": np.ascontiguousarray(bm_all[shard]).reshape(1, MODS),
            "w_qkv": wq, "w_o": wo, "w_1": w1, "w_2": w2,
            "l_slot": np.array([[(c - 1) % NC_N]], np.uint32),
            "m_glob": mg, "m_loc": np.ascontiguousarray(ml),
            "rope_t": rope,
        })
    return in_maps


def kernel(**inputs):
    if 'nc' not in _CACHE:
        _CACHE['nc'] = build()
    nc = _CACHE['nc']
    in_maps = _prep_inputs(inputs)
    res = bass_utils.run_bass_kernel_spmd(
        nc, in_maps, core_ids=list(range(NC_N)),
        trace=bool(int(os.environ.get("DIT_TRACE", "0"))))
    _CACHE['last_result'] = res
    out = np.concatenate([res.results[c]["x_out"] for c in range(NC_N)], axis=0)
    return out.reshape(B, S, D).astype(np.float32)


if __name__ == "__main__":
    import reference
    inputs = reference.setup_inputs()
    expected = np.asarray(reference.reference(**inputs))
    actual = kernel(**{k: np.asarray(v) for k, v in inputs.items()})
    rel = np.linalg.norm(actual - expected) / np.linalg.norm(expected)
    print("max abs err:", np.abs(actual - expected).max())
    print("Relative error:", rel)
    for c in range(NC_N):
        a, e = actual[0, c * T:(c + 1) * T], expected[0, c * T:(c + 1) * T]
        r = np.linalg.norm(a - e) / np.linalg.norm(e)
        print(f"  core {c}: rel={r:.3e} maxabs={np.abs(a - e).max():.3e} "
              f"nan={np.isnan(a).sum()}")
